# revision 12
# baseline (speedup 1.0000x reference)
"""CharacterCNNEmbedding Trainium2 Bass kernel.

Full inputs -> full output [8, 512, 2048]. Data-parallel over 8 NeuronCores
(512 words each). Host marshals the char embedding gather into a merged
conv moving matrix mov[128, 512*56] fp16 (112 window rows = 7 taps x 16
chans, +1 ones row folding conv bias, padded to 128 partitions for the
fast HWDGE descriptor path; shipped as 16 contiguous column chunks).
Per core:
  - one fp16 matmul per (filter-tile, 8-word chunk), K=128, evaluates all
    7 conv widths at 44 positions; 17 masked tail matmuls cover positions
    44..49 for widths < 7
  - relu+max-pool split two ways to balance engines: DVE-direct m-tiles
    (tensor_reduce from PSUM) and ACT-staged m-tiles (relu->fp16 staging,
    DVE overlap-fold tree 44->22->12->6->reduce at 2x perf mode)
  - 2 highway layers + projection as fp16 2048x2048 matmuls; weights
    streamed as one [128, 2048] strip DMA per (matrix, m-tile); ACT
    relu/sigmoid with fused bias, DVE gate combine
"""
import sys

sys.path.insert(0, "/opt/trn_rl_repo")
import numpy as np

B, S, L = 8, 512, 50
LP = 56                      # padded word length (taps stay in-word)
NCORES = 8
W = 512                      # words per core
QW = 128                     # words per quarter
NQ = W // QW
COLS = W * LP                # 28672
NCK = 16                     # mov ingest chunks
CKC = COLS // NCK            # 1792 cols per chunk
CD = 16
TOTAL_F = 2048
NMT = TOTAL_F // 128         # 16 filter tiles
CHW = 8                      # conv chunk words (352 cols)
NCH = QW // CHW              # 16 chunks per quarter
GRP = 2                      # chunks per PSUM group
SPL = 12                     # positions 0:SPL drain via DVE reduce, SPL:44 via ACT
FILTERS = [(1, 32), (2, 32), (3, 64), (4, 128), (5, 256), (6, 512), (7, 1024)]
TAIL_TILES = (
    [(44, j) for j in range(8)]
    + [(45, j) for j in range(4)]
    + [(46, j) for j in range(2)]
    + [(47, 0), (48, 0), (49, 0)]
)

_prog_cache = {}


def _build_program(n_words=W):
    import concourse.tile as tile
    from concourse import bacc, mybir

    F32 = mybir.dt.float32
    F16 = mybir.dt.float16
    nq = n_words // QW
    cols = n_words * LP

    nc = bacc.Bacc("TRN2", target_bir_lowering=False, debug=False)

    mov_d = nc.dram_tensor("mov", [NCK, 128, CKC], F16, kind="ExternalInput").ap()
    convw_d = nc.dram_tensor("convw", [128, TOTAL_F], F16, kind="ExternalInput").ap()
    tailw_d = nc.dram_tensor(
        "tailw", [128, len(TAIL_TILES) * 128], F16, kind="ExternalInput"
    ).ap()
    wbig_d = nc.dram_tensor(
        "wbig", [5, NMT, 128, TOTAL_F], F16, kind="ExternalInput"
    ).ap()
    hbT_d = nc.dram_tensor("hbT", [128, 5 * NMT], F32, kind="ExternalInput").ap()
    out_d = nc.dram_tensor("out", [TOTAL_F, n_words], F32, kind="ExternalOutput").ap()

    AF = mybir.ActivationFunctionType
    OP = mybir.AluOpType
    AX = mybir.AxisListType

    with tile.TileContext(nc) as tc:
        with (
            tc.tile_pool(name="const", bufs=1) as cpool,
            tc.tile_pool(name="h", bufs=1) as hp,
            tc.tile_pool(name="ws", bufs=3) as wsp,
            tc.tile_pool(name="tmp", bufs=2) as tmpp,
            tc.tile_pool(name="outp", bufs=2) as outp,
        ):
            mov_sb = cpool.tile([128, cols], F16)
            for c in range(NCK):
                nc.sync.dma_start(mov_sb[:, c * CKC : (c + 1) * CKC], mov_d[c])
            convw_sb = cpool.tile([128, TOTAL_F], F16)
            nc.sync.dma_start(convw_sb[:], convw_d[:])
            tailw_sb = cpool.tile([128, len(TAIL_TILES) * 128], F16)
            nc.sync.dma_start(tailw_sb[:], tailw_d[:])
            hbT_sb = cpool.tile([128, 5 * NMT], F32)
            nc.sync.dma_start(hbT_sb[:], hbT_d[:])
            movv = mov_sb.rearrange("p (w l) -> p w l", l=LP)

            h0 = [hp.tile([128, W], F16, name=f"ha_{k}") for k in range(NMT)]

            # ---------------- conv + pool phase ----------------
            with (
                tc.tile_pool(name="stg", bufs=3) as stgp,
                tc.tile_pool(name="fold", bufs=3) as foldp,
                tc.tile_pool(name="convp", bufs=4, space="PSUM") as convpp,
            ):
                tails_by_j = {}
                for idx, (tt, j) in enumerate(TAIL_TILES):
                    tails_by_j.setdefault(j, []).append((idx, tt))

                for q in range(nq):
                    for m in range(NMT):
                        wsl = convw_sb[:, m * 128 : (m + 1) * 128]
                        # DVE max over raw positions 0:SPL, per group
                        rm = foldp.tile([128, QW], F16, name="rm")
                        # ACT relu-staged positions SPL:44
                        stg = stgp.tile([128, QW, 44 - SPL], F16, name="stg")
                        ci = 0
                        while ci < NCH:
                            ng = min(GRP, NCH - ci)
                            cp = convpp.tile([128, GRP, 512], F32, name="cv")
                            for i in range(ng):
                                w0 = q * QW + (ci + i) * CHW
                                nc.tensor.matmul(
                                    cp[:, i, 0 : CHW * 44],
                                    wsl,
                                    movv[:, w0 : w0 + CHW, 0:44],
                                    start=True,
                                    stop=True,
                                )
                            src4 = cp[:, 0:ng, 0 : CHW * 44].rearrange(
                                "p c (w l) -> p c w l", l=44
                            )
                            nc.vector.tensor_reduce(
                                rm[:, ci * CHW : (ci + ng) * CHW].rearrange(
                                    "p (c w) -> p c w", c=ng
                                ),
                                src4[:, :, :, 0:SPL],
                                op=OP.max,
                                axis=AX.X,
                            )
                            nc.scalar.activation(
                                stg[:, ci * CHW : (ci + ng) * CHW, :].rearrange(
                                    "p (c w) l -> p c w l", c=ng
                                ),
                                src4[:, :, :, SPL:44],
                                AF.Relu,
                            )
                            ci += ng
                        # fold tree over the 32 staged positions (fp16, 2x mode)
                        f16t = foldp.tile([128, QW, 16], F16, name="f16t")
                        nc.vector.tensor_max(
                            f16t[:], stg[:, :, 0:16], stg[:, :, 16:32]
                        )
                        f8t = foldp.tile([128, QW, 8], F16, name="f8t")
                        nc.vector.tensor_max(
                            f8t[:], f16t[:, :, 0:8], f16t[:, :, 8:16]
                        )
                        f4t = foldp.tile([128, QW, 4], F16, name="f4t")
                        nc.vector.tensor_max(f4t[:], f8t[:, :, 0:4], f8t[:, :, 4:8])
                        s4 = foldp.tile([128, QW], F16, name="s4")
                        nc.vector.tensor_reduce(s4[:], f4t, op=OP.max, axis=AX.X)
                        # h = relu(max(raw, relu'd)) = (rm max 0) max s4
                        hslice = h0[m][:, q * QW : (q + 1) * QW]
                        nc.vector.scalar_tensor_tensor(
                            hslice, rm[:], 0.0, s4[:], op0=OP.max, op1=OP.max
                        )
                        # tails for this j-tile, interleaved into the last
                        # quarter so they overlap remaining pool work
                        if q == nq - 1:
                            for idx, tt in tails_by_j.get(m, []):
                                tp = convpp.tile([128, GRP, 512], F32, name="cv")
                                nc.tensor.matmul(
                                    tp[:, 0, 0:n_words],
                                    tailw_sb[:, idx * 128 : (idx + 1) * 128],
                                    movv[:, :, tt],
                                    start=True,
                                    stop=True,
                                )
                                nc.vector.tensor_max(
                                    h0[m][:], h0[m][:], tp[:, 0, 0:n_words]
                                )

            # ---------------- highway + projection ----------------
            with tc.tile_pool(name="hw", bufs=4, space="PSUM") as hwp:
                hin = h0
                for layer in range(2):
                    hout = [
                        hp.tile([128, W], F16, name=f"h{'b' if layer == 0 else 'a'}_{k}")
                        for k in range(NMT)
                    ]
                    for m in range(NMT):
                        wsT = wsp.tile([128, TOTAL_F], F16, name="wsT")
                        nc.sync.dma_start(wsT[:], wbig_d[2 * layer, m])
                        wsG = wsp.tile([128, TOTAL_F], F16, name="wsG")
                        nc.sync.dma_start(wsG[:], wbig_d[2 * layer + 1, m])
                        pt = hwp.tile([128, 512], F32, name="pt")
                        pg = hwp.tile([128, 512], F32, name="pg")
                        for k in range(NMT):
                            nc.tensor.matmul(
                                pt[:, 0:n_words],
                                wsT[:, k * 128 : (k + 1) * 128],
                                hin[k][:],
                                start=(k == 0),
                                stop=(k == NMT - 1),
                            )
                        for k in range(NMT):
                            nc.tensor.matmul(
                                pg[:, 0:n_words],
                                wsG[:, k * 128 : (k + 1) * 128],
                                hin[k][:],
                                start=(k == 0),
                                stop=(k == NMT - 1),
                            )
                        t_sb = tmpp.tile([128, W], F16, name="t_sb")
                        nc.scalar.activation(
                            t_sb[:],
                            pt[:, 0:n_words],
                            AF.Relu,
                            bias=hbT_sb[:, 2 * layer * NMT + m : 2 * layer * NMT + m + 1],
                        )
                        g_sb = tmpp.tile([128, W], F16, name="g_sb")
                        nc.scalar.activation(
                            g_sb[:],
                            pg[:, 0:n_words],
                            AF.Sigmoid,
                            bias=hbT_sb[
                                :, (2 * layer + 1) * NMT + m : (2 * layer + 1) * NMT + m + 1
                            ],
                        )
                        d_sb = tmpp.tile([128, W], F16, name="de")
                        nc.vector.tensor_sub(d_sb[:], t_sb[:], hin[m][:])
                        e_sb = tmpp.tile([128, W], F16, name="de")
                        nc.vector.tensor_mul(e_sb[:], g_sb[:], d_sb[:])
                        nc.vector.tensor_add(hout[m][:], hin[m][:], e_sb[:])
                    hin = hout

                for m in range(NMT):
                    wsP = wsp.tile([128, TOTAL_F], F16, name="wsT")
                    nc.sync.dma_start(wsP[:], wbig_d[4, m])
                    pp = hwp.tile([128, 512], F32, name="pt")
                    for k in range(NMT):
                        nc.tensor.matmul(
                            pp[:, 0:n_words],
                            wsP[:, k * 128 : (k + 1) * 128],
                            hin[k][:],
                            start=(k == 0),
                            stop=(k == NMT - 1),
                        )
                    o_sb = outp.tile([128, W], F32, name="o_sb")
                    nc.scalar.activation(
                        o_sb[:], pp[:, 0:n_words], AF.Identity,
                        bias=hbT_sb[:, 4 * NMT + m : 4 * NMT + m + 1],
                    )
                    nc.sync.dma_start(out_d[m * 128 : (m + 1) * 128, :], o_sb[:])

    nc.compile()
    return nc


def _prep_weights(inputs):
    """Host-side weight marshalling (layout + fp16 rounding)."""
    f32 = np.float32
    convw = np.zeros((128, TOTAL_F), f32)
    offs = np.concatenate([[0], np.cumsum([nf for _, nf in FILTERS])])
    widths = np.repeat([w for w, _ in FILTERS], [nf for _, nf in FILTERS])
    for i, (w, nf) in enumerate(FILTERS):
        cw = np.asarray(inputs[f"conv_w{i}"], f32)  # [nf, 16, w]
        for dt in range(w):
            convw[dt * CD : (dt + 1) * CD, offs[i] : offs[i] + nf] = cw[:, :, dt].T
        convw[112, offs[i] : offs[i] + nf] = np.asarray(inputs[f"conv_b{i}"], f32)
    tailw = np.zeros((128, len(TAIL_TILES) * 128), f32)
    for idx, (tt, j) in enumerate(TAIL_TILES):
        blk = convw[:, 128 * j : 128 * (j + 1)].copy()
        blk[:, widths[128 * j : 128 * (j + 1)] > (50 - tt)] = 0.0
        tailw[:, 128 * idx : 128 * (idx + 1)] = blk

    wstack = np.stack(
        [
            np.asarray(inputs["hw0_tw"], f32).T,
            np.asarray(inputs["hw0_gw"], f32).T,
            np.asarray(inputs["hw1_tw"], f32).T,
            np.asarray(inputs["hw1_gw"], f32).T,
            np.asarray(inputs["proj_w"], f32).T,
        ]
    ).astype(np.float16)
    # strip layout: wbig[l, m][p, k*128+j] = W.T[l][k*128+p, m*128+j]
    wbig = np.ascontiguousarray(
        wstack.reshape(5, NMT, 128, NMT, 128)
        .transpose(0, 3, 2, 1, 4)
        .reshape(5, NMT, 128, TOTAL_F)
    )
    hb = [
        np.asarray(inputs["hw0_tb"], f32),
        np.asarray(inputs["hw0_gb"], f32),
        np.asarray(inputs["hw1_tb"], f32),
        np.asarray(inputs["hw1_gb"], f32),
        np.asarray(inputs["proj_b"], f32),
    ]
    hbT = np.zeros((128, 5 * NMT), f32)
    for p_i in range(5):
        for m in range(NMT):
            hbT[:, p_i * NMT + m] = hb[p_i][m * 128 : (m + 1) * 128]

    return {
        "convw": convw.astype(np.float16),
        "tailw": tailw.astype(np.float16),
        "wbig": wbig,
        "hbT": hbT,
    }


def _prep_mov(char_ids, char_table):
    """Gather char embeddings and build the per-core merged conv moving
    matrix mov[128, W*LP]: row 16*dt+c, col w*LP+l = emb[w, l+dt, c];
    row 112 = ones (bias), rows 113..127 zero pad. Shipped chunk-major
    [NCK, 128, CKC] so each ingest DMA is a contiguous [128, CKC] block."""
    table = np.asarray(char_table, np.float32).copy()
    table[0] = 0.0
    table16 = table.astype(np.float16)
    ids = np.asarray(char_ids).reshape(B * S, L).astype(np.int64)
    per_core = []
    for c in range(NCORES):
        idsc = ids[c * W : (c + 1) * W]                      # [W, L]
        embp = np.zeros((W, LP + 7, CD), np.float16)
        embp[:, :L, :] = table16[idsc]
        mov = np.zeros((128, W * LP), np.float16)
        for dt in range(7):
            mov[16 * dt : 16 * dt + 16, :] = (
                embp[:, dt : dt + LP, :].transpose(2, 0, 1).reshape(CD, W * LP)
            )
        mov[112, :] = 1.0
        movc = np.ascontiguousarray(
            mov.reshape(128, NCK, CKC).transpose(1, 0, 2)
        )
        per_core.append(movc)
    return per_core


def _run(inputs, trace=False):
    from concourse.bass_utils import run_bass_kernel_spmd

    if "prog" not in _prog_cache:
        _prog_cache["prog"] = _build_program()
    nc = _prog_cache["prog"]

    shared = _prep_weights(inputs)
    movs = _prep_mov(inputs["char_ids"], inputs["char_table"])
    in_maps = [dict(shared, mov=movs[c]) for c in range(NCORES)]
    br = run_bass_kernel_spmd(nc, in_maps, list(range(NCORES)), trace=trace)
    outs = [br.results[c]["out"] for c in range(NCORES)]  # [2048, 512] each
    full = np.concatenate([o.T for o in outs], axis=0)  # [4096, 2048]
    return full.reshape(B, S, TOTAL_F).astype(np.float32), br


def kernel(**inputs):
    out, _ = _run(inputs, trace=False)
    return out
